# revision 13
# baseline (speedup 1.0000x reference)
"""Multi-head self-attention (B=4, T=2048, D=1024, H=16) on 8 TRN2 NeuronCores.

Reference quirk: softmax normalizes over the QUERY axis (dim=2 of
[B,H,T1,T2]), i.e. attn[q,k] = exp(s[q,k]) / sum_q' exp(s[q',k]).

Sharding (fully SPMD, one NEFF for all 8 cores):
  core c -> batch b = c//2, head-group g = c%2 (8 heads = 512 cols of Wq/Wk/Wv).
  Host pre-slices AND pre-transposes per-core inputs (xT, wqT/wkT/wvT), runs
  the kernel, and stitches the 8 transposed [E, T] output shards back.

Device algorithm per core:
  1. QT/KT [128e, T] per head-pair (partition = head dims of 2 heads),
     V [128t, 512e] natural, via PE from xT / w*T tiles (fp32r).
  2. Per head-pair, per 128-wide key chunk:
       S' = K @ Q^T chunk [128 k, T q] in PSUM (row-tiled pair: head A rows
       0-63, head B rows 64-127, concurrent).
       exp: 3 of 4 [128,1024] tiles via ScalarE ACT (accum_out = Z partials);
       the 4th tile (head A, qb0) via the Vector engine: Schraudolph bit-trick
       (tensor_scalar fp32->int32 round-convert) + a custom 7-stage DVE op
       that polynomial-corrects the mantissa (max rel err ~0.54%) and
       accumulates the Z partial. This offloads ~25% of the exp stream from
       the Scalar engine (the serial bottleneck).
       V'[k,:] = V[k,:] / Z[k] (normalization folded into V, scaled on gpsimd),
       outT[d, q] += matmul: lhsT=V'_h [128k, 64d] (col tile_position per
       head, no zero padding), rhs=P [128k, 512q], accumulated in PSUM.
  3. Epilogue: acc -> SBUF -> DRAM as outT [E, T]; final transpose on host.

Pipeline: per chunk the PE stream is S -> projection fillers -> AV, so the PE
never idles long enough for the HAM clock gate to re-throttle.
"""

import operator

import numpy as np

B, T, D, H = 4, 2048, 1024, 16
DH = D // H
SCALE = 1.0 / (DH**0.5)
N_CORES = 8
E = D // 2  # 512 output cols per core (8 heads)
N_PAIRS = 4  # head-pairs per core
N_DC = D // 128  # 8 contraction chunks for projections
N_KC = T // 128  # 16 key chunks
QB = 1024  # exp free-dim block (2 PSUM banks)

# Schraudolph exp constants: z = round(2^23 * (log2e*SCALE*s + 127)),
# bitcast to fp32 gives 2^i*(1+f); custom DVE op multiplies by
# (1 + QC*f*(f-1)) to correct the mantissa interpolation.
A_SCHRAUD = float((2.0**23) * np.log2(np.e) * SCALE)
B_SCHRAUD = 127.0 * (2.0**23)
MASK_VAL = float(np.int32(0x007FFFFF).view(np.float32))  # +subnormal mantissa mask
TWO_P126 = float(np.float32(2.0**126))
QC = 0.23547743862603948

_built = None  # (nc,) cache so repeat kernel() calls skip rebuild/recompile
_exp_op = None


def _np_reference(x, padding_mask, Wq, Wk, Wv):
    """Pure-numpy fallback, used only if the mask is not all-ones."""
    x64 = x.astype(np.float64)
    Q = (x64 @ Wq.T.astype(np.float64)).reshape(B, T, H, DH).transpose(0, 2, 1, 3)
    K = (x64 @ Wk.T.astype(np.float64)).reshape(B, T, H, DH).transpose(0, 2, 1, 3)
    V = (x64 @ Wv.T.astype(np.float64)).reshape(B, T, H, DH).transpose(0, 2, 1, 3)
    s = np.einsum("bhqd,bhkd->bhqk", Q, K) * SCALE
    s = np.where(padding_mask[:, None, :, :] == 0, -np.inf, s)
    s = s - s.max(axis=2, keepdims=True)
    p = np.exp(s)
    p = p / p.sum(axis=2, keepdims=True)
    out = np.einsum("bhqk,bhkd->bhqd", p, V)
    return out.transpose(0, 2, 1, 3).reshape(B, T, D).astype(np.float32)


def _get_exp_op():
    """Register (once) the custom DVE op: out = z0*(1 + QC*f*(f-1)) with
    f extracted from z0's mantissa bits, plus a running sum (accum_out)."""
    global _exp_op
    if _exp_op is not None:
        return _exp_op
    import concourse.dve_ops as dve_ops_mod
    from concourse.dve_ops import DveOp
    from concourse.dve_spec import AluOp, Bin, C0, C1, C2, One, Spec, Src0, Zero, lower
    from concourse.dve_uop import DveOpSpec

    _a = Bin(AluOp.BITWISE_AND, Src0, C0)  # mantissa bits as +subnormal
    _d = _a * C1  # f in [0,1)  (subnormal * 2^126)
    _e = _d - One
    _u = _d * _e
    _v = _u * C2
    _t = _v * Src0
    spec = Spec(body=Src0 + _t, accum=operator.add, accum_init=Zero)

    name = "EXP_SFIX_ANT"
    existing = next((o for o in dve_ops_mod.OPS if o.name == name), None)
    if existing is not None:
        _exp_op = existing
        return existing
    sha = DveOpSpec(name=name, opcode=0, uops=lower(spec, ver="v3"), rd1_en=False).sha(
        "v3"
    )
    op = DveOp(name, spec, subdim=False, uops_sha={"v3": sha})
    dve_ops_mod.OPS.append(op)
    dve_ops_mod._SUB_OPCODE_FOR_NAME[name] = (
        dve_ops_mod._CUSTOM_DVE_ROW_BASE + len(dve_ops_mod.OPS) - 1
    )
    _exp_op = op
    return op


def _split_multi_waits(nc):
    """Walrus caps sync waits at 1 per instruction; Tile's tail drain can carry
    several. Move the extras onto single-wait drains appended to the previous
    basic block (same engine, earlier in program order)."""
    import concourse.mybir as mybir

    blocks = list(nc.m.functions[0].blocks)
    for bi, blk in enumerate(blocks):
        for inst in blk.instructions:
            if type(inst).__name__ not in ("InstDrain", "InstNoOp", "InstEventSemaphore"):
                continue
            si = inst.sync_info
            if si is not None and si.on_wait and len(si.on_wait) > 1:
                waits = list(si.on_wait)
                keep, extra = waits[-1], waits[:-1]
                assert all(w.wait_mode == "sem-ge-imm" for w in extra), extra
                si.on_wait = [keep]
                assert bi > 0, "multi-wait in first block"
                prev = blocks[bi - 1]
                for j, w in enumerate(extra):
                    d = mybir.InstDrain(
                        name=f"{inst.name}-ws{j}",
                        engine=inst.engine,
                        sync_info=mybir.SyncInfo(on_wait=[w], on_update=[]),
                    )
                    prev.add_instruction(d)


def _build_kernel(tc, xT, wqT, wkT, wvT, outT):
    import concourse.bass as bass  # noqa: F401
    import concourse.mybir as mybir

    nc = tc.nc
    FP = mybir.dt.float32
    FR = mybir.dt.float32r
    BF = mybir.dt.bfloat16
    I32 = mybir.dt.int32
    Exp = mybir.ActivationFunctionType.Exp
    exp_op = _get_exp_op()

    # long-lived pools
    xw = tc.alloc_tile_pool(name="xw", bufs=1)
    wp = tc.alloc_tile_pool(name="wp", bufs=3)
    qkv = tc.alloc_tile_pool(name="qkv", bufs=1)
    # PSUM: S pool (2x [128,QB] = 4 banks) shared by scores and projections;
    # acc pool (2x [128,QB] = 4 banks) for outT accumulation.
    sps = tc.alloc_tile_pool(name="sps", bufs=2, space="PSUM")
    accps = tc.alloc_tile_pool(name="accps", bufs=1, space="PSUM")
    pp = tc.alloc_tile_pool(name="pp", bufs=8)
    zp = tc.alloc_tile_pool(name="zp", bufs=4)
    z0p = tc.alloc_tile_pool(name="z0p", bufs=2)
    vpp = tc.alloc_tile_pool(name="vpp", bufs=4)
    op = tc.alloc_tile_pool(name="op", bufs=2)

    # ---- loads (interleaved so projection accumulation can start early) ----
    xTs = [None] * N_DC
    wq, wk, wv = [None] * N_DC, [None] * N_DC, [None] * N_DC
    for dc in range(N_DC):
        t = xw.tile([128, T], FR, name=f"xT{dc}", tag=f"x{dc}")
        nc.sync.dma_start(out=t, in_=xT[dc * 128 : (dc + 1) * 128, :])
        xTs[dc] = t
        for ws, wap, label in ((wq, wqT, "wq"), (wk, wkT, "wk"), (wv, wvT, "wv")):
            wt = wp.tile([128, E], FR, name=f"{label}{dc}", tag=f"w{dc}")
            nc.sync.dma_start(out=wt, in_=wap[dc * 128 : (dc + 1) * 128, :])
            ws[dc] = wt

    # ---- projection emitters (psum borrowed from the S pool tag) ----
    copy_flip = [0]
    ramp = [True]  # during the upfront ramp ScalarE is idle; share copies

    def _proj_copy(dst, src):
        """PSUM->SBUF projection copies: alternate ScalarE/VectorE during the
        upfront ramp (ACT idle there); VectorE only in steady state (ACT is
        the bottleneck engine then)."""
        if ramp[0] and copy_flip[0] % 2 == 0:
            nc.scalar.copy(dst, src)
        else:
            nc.vector.tensor_copy(dst, src)
        copy_flip[0] += 1

    def project_eT_tile(ws, pair, tt, et):
        """One [128, 512] t-block of QT/KT pair tile `et` (bf16 [128, T])."""
        ps = sps.tile([128, QB], FP, name=f"ps_{et.tensor.name}_{tt}", tag="s")
        for dc in range(N_DC):
            nc.tensor.matmul(
                ps[:, 0:512],
                ws[dc][:, pair * 128 : (pair + 1) * 128],
                xTs[dc][:, tt * 512 : (tt + 1) * 512],
                start=(dc == 0),
                stop=(dc == N_DC - 1),
            )
        _proj_copy(et[:, tt * 512 : (tt + 1) * 512], ps[:, 0:512])

    def project_v_tile(tt):
        v = qkv.tile([128, E], BF, name=f"v{tt}", tag=f"v{tt}")
        ps = sps.tile([128, QB], FP, name=f"ps_v{tt}", tag="s")
        for dc in range(N_DC):
            nc.tensor.matmul(
                ps[:, 0:512],
                xTs[dc][:, tt * 128 : (tt + 1) * 128],
                wv[dc],
                start=(dc == 0),
                stop=(dc == N_DC - 1),
            )
        _proj_copy(v, ps[:, 0:512])
        return v

    QT = [None] * N_PAIRS
    KT = [None] * N_PAIRS
    V = [None] * N_KC

    # pair-0 QT/KT + V[0] upfront (ramp); V[c] and later pairs' QT/KT spread
    # into the chunk stream as PE fillers (keeps the HAM clock gate warm).
    for pair in range(N_PAIRS):
        QT[pair] = qkv.tile([128, T], BF, name=f"qt{pair}", tag=f"qt{pair}")
        KT[pair] = qkv.tile([128, T], BF, name=f"kt{pair}", tag=f"kt{pair}")
    # chunk c of pair 0 needs all of QT[0] but only KT piece c//4: emit
    # KT0-tt0 first, then QT0, then V0; KT0-tt1..3 become early fillers.
    project_eT_tile(wk, 0, 0, KT[0])
    for tt in range(4):
        project_eT_tile(wq, 0, tt, QT[0])
    V[0] = project_v_tile(0)
    ramp[0] = False

    # pair p+1's 8 QT/KT pieces spread over every other chunk of pair p
    FILLER_CHUNKS = list(range(0, 16, 2))

    for p in range(N_PAIRS):
        acc = [
            accps.tile([128, QB], FP, name=f"acc{qb}_{p}", tag=f"acc{qb}")
            for qb in range(2)
        ]
        pending_av = None
        for c in range(N_KC):
            kt_lo = KT[p][0:64, c * 128 : (c + 1) * 128]
            kt_hi = KT[p][64:128, c * 128 : (c + 1) * 128]

            def s_mm(dst, kt, base, q0):
                nc.tensor.matmul(
                    dst,
                    kt,
                    QT[p][base : base + 64, q0 : q0 + 512],
                    start=True,
                    stop=True,
                    tile_position=(base, 0),
                )

            def av_mm(hi, qb, qt, cc, vts_, pt_):
                nc.tensor.matmul(
                    acc[qb][:, qt * 512 : (qt + 1) * 512],
                    vts_[hi],
                    pt_[(hi, qb)][:, qt * 512 : (qt + 1) * 512],
                    start=(cc == 0 and hi == 0),
                    stop=(cc == N_KC - 1 and hi == 1),
                )

            # ---- scores qb0 (row-tiled concurrent pair) ----
            s0A = sps.tile([128, QB], FP, name=f"s_{p}_{c}_A0", tag="s")
            s0B = sps.tile([128, QB], FP, name=f"s_{p}_{c}_B0", tag="s")
            for qt in range(2):
                s_mm(s0A[:, qt * 512 : qt * 512 + 512], kt_lo, 0, qt * 512)
                s_mm(s0B[:, qt * 512 : qt * 512 + 512], kt_hi, 64, qt * 512)
            zs = zp.tile([128, 4], FP, name=f"zs_{p}_{c}", tag="zs")
            # head A qb0 -> Vector engine: Schraudolph int-convert (two halves
            # so s0A's PSUM buf frees as early as possible) + custom fixup.
            z0t = z0p.tile([128, QB], FP, name=f"z0_{p}_{c}", tag="z0")
            for h in range(2):
                nc.vector.tensor_scalar(
                    out=z0t.bitcast(I32)[:, h * 512 : (h + 1) * 512],
                    in0=s0A[:, h * 512 : (h + 1) * 512],
                    scalar1=A_SCHRAUD,
                    scalar2=B_SCHRAUD,
                    op0=mybir.AluOpType.mult,
                    op1=mybir.AluOpType.add,
                )
            pA0 = pp.tile([128, QB], BF, name=f"p_{p}_{c}_A0", tag="p")
            nc.vector._custom_dve(
                exp_op,
                out=pA0,
                in0=z0t,
                s0=MASK_VAL,
                s1=TWO_P126,
                imm2=QC,
                accum_out=zs[:, 0:1],
            )
            # head B qb0 -> Scalar engine
            pB0 = pp.tile([128, QB], BF, name=f"p_{p}_{c}_B0", tag="p")
            nc.scalar.activation(
                out=pB0, in_=s0B, func=Exp, scale=SCALE, accum_out=zs[:, 2:3]
            )
            # previous chunk's AV, first half (fills the PE while exp drains)
            if pending_av is not None:
                pc, pvts, ppt = pending_av
                for qt in range(2):
                    av_mm(0, 0, qt, pc, pvts, ppt)
                    av_mm(0, 1, qt, pc, pvts, ppt)
            # ---- scores qb1: B first (reuses s0A's buf, freed by op1) ----
            s1B = sps.tile([128, QB], FP, name=f"s_{p}_{c}_B1", tag="s")
            for qt in range(2):
                s_mm(s1B[:, qt * 512 : qt * 512 + 512], kt_hi, 64, QB + qt * 512)
            pB1 = pp.tile([128, QB], BF, name=f"p_{p}_{c}_B1", tag="p")
            nc.scalar.activation(
                out=pB1, in_=s1B, func=Exp, scale=SCALE, accum_out=zs[:, 3:4]
            )
            s1A = sps.tile([128, QB], FP, name=f"s_{p}_{c}_A1", tag="s")
            for qt in range(2):
                s_mm(s1A[:, qt * 512 : qt * 512 + 512], kt_lo, 0, QB + qt * 512)
            pA1 = pp.tile([128, QB], BF, name=f"p_{p}_{c}_A1", tag="p")
            nc.scalar.activation(
                out=pA1, in_=s1A, func=Exp, scale=SCALE, accum_out=zs[:, 1:2]
            )
            ptiles = {(0, 0): pA0, (0, 1): pA1, (1, 0): pB0, (1, 1): pB1}
            # previous chunk's AV, second half
            if pending_av is not None:
                pc, pvts, ppt = pending_av
                for qt in range(2):
                    av_mm(1, 0, qt, pc, pvts, ppt)
                    av_mm(1, 1, qt, pc, pvts, ppt)
            # ---- Z = qb0 + qb1 partial sums (gpsimd); r = 1/Z; V' = V*r ----
            za = zp.tile([128, 2], FP, name=f"za_{p}_{c}", tag="za")
            nc.gpsimd.tensor_add(za[:, 0:1], zs[:, 0:1], zs[:, 1:2])
            nc.gpsimd.tensor_add(za[:, 1:2], zs[:, 2:3], zs[:, 3:4])
            rz = zp.tile([128, 2], FP, name=f"rz_{p}_{c}", tag="rz")
            nc.vector.reciprocal(out=rz, in_=za)
            vts = []
            for hi in range(2):
                vt = vpp.tile([128, 128], BF, name=f"vp{hi}_{p}_{c}", tag=f"vp{hi}")
                lo, hi_ = (0, 64) if hi == 0 else (64, 128)
                zlo, zhi = (64, 128) if hi == 0 else (0, 64)
                nc.gpsimd.memset(vt[:, zlo:zhi], 0.0)
                nc.vector.tensor_scalar_mul(
                    vt[:, lo:hi_],
                    V[c][:, p * 128 + lo : p * 128 + hi_],
                    rz[:, hi : hi + 1],
                )
                vts.append(vt)
            pending_av = (c, vts, ptiles)
            # ---- fillers at slot end: their PSUM piece lands in the "s"
            # rotation after this chunk's 4 tiles; the copy runs early in the
            # next slot. ----
            if p == 0 and c < 3:
                project_eT_tile(wk, 0, c + 1, KT[0])
            if p == 0 and c + 1 < N_KC:
                V[c + 1] = project_v_tile(c + 1)
            if p < N_PAIRS - 1 and c in FILLER_CHUNKS:
                idx = FILLER_CHUNKS.index(c)
                if idx < 4:
                    project_eT_tile(wq, p + 1, idx, QT[p + 1])
                else:
                    project_eT_tile(wk, p + 1, idx - 4, KT[p + 1])
        pc, pvts, ppt = pending_av
        for hi in range(2):
            for qt in range(2):
                av_mm(hi, 0, qt, pc, pvts, ppt)
                av_mm(hi, 1, qt, pc, pvts, ppt)
        # epilogue: outT rows for this pair -> SBUF -> DRAM (host transposes)
        for qb in range(2):
            ot = op.tile([128, QB], FP, name=f"ot_{p}_{qb}", tag="ot")
            nc.vector.tensor_copy(ot, acc[qb])
            nc.sync.dma_start(
                out=outT[p * 128 : (p + 1) * 128, qb * QB : (qb + 1) * QB],
                in_=ot,
            )

    for pool in (op, vpp, z0p, zp, pp, accps, sps, qkv, wp, xw):
        pool.release()


def build():
    import concourse.bacc as bacc
    import concourse.mybir as mybir
    import concourse.tile as tile

    nc = bacc.Bacc("TRN2", target_bir_lowering=False, debug=False)
    FP = mybir.dt.float32
    FR = mybir.dt.float32r
    xT = nc.dram_tensor("xT", [D, T], FR, kind="ExternalInput").ap()
    wqT = nc.dram_tensor("wqT", [D, E], FR, kind="ExternalInput").ap()
    wkT = nc.dram_tensor("wkT", [D, E], FR, kind="ExternalInput").ap()
    wvT = nc.dram_tensor("wvT", [D, E], FR, kind="ExternalInput").ap()
    outT = nc.dram_tensor("outT", [E, T], FP, kind="ExternalOutput").ap()
    with tile.TileContext(nc) as tc:
        _build_kernel(tc, xT, wqT, wkT, wvT, outT)
    nc.compile()
    _split_multi_waits(nc)
    return nc


def _get_nc():
    global _built
    if _built is None:
        _built = build()
    return _built


def make_in_maps(x, Wq, Wk, Wv):
    in_maps = []
    for c in range(N_CORES):
        b, g = divmod(c, 2)
        e0 = E * g
        in_maps.append(
            {
                "xT": np.ascontiguousarray(x[b].T),
                "wqT": np.ascontiguousarray(Wq[e0 : e0 + E, :].T),
                "wkT": np.ascontiguousarray(Wk[e0 : e0 + E, :].T),
                "wvT": np.ascontiguousarray(Wv[e0 : e0 + E, :].T),
            }
        )
    return in_maps


def assemble_out(results):
    out = np.empty((B, T, D), np.float32)
    for c in range(N_CORES):
        b, g = divmod(c, 2)
        e0 = E * g
        out[b][:, e0 : e0 + E] = results[c]["outT"].T
    return out


def kernel(x, padding_mask, Wq, Wk, Wv):
    x = np.asarray(x, dtype=np.float32)
    padding_mask = np.asarray(padding_mask, dtype=np.float32)
    Wq = np.asarray(Wq, dtype=np.float32)
    Wk = np.asarray(Wk, dtype=np.float32)
    Wv = np.asarray(Wv, dtype=np.float32)
    if not np.all(padding_mask == 1.0):
        return _np_reference(x, padding_mask, Wq, Wk, Wv)

    from concourse.bass_utils import run_bass_kernel_spmd

    nc = _get_nc()
    in_maps = make_in_maps(x, Wq, Wk, Wv)
    res = run_bass_kernel_spmd(nc, in_maps, list(range(N_CORES)))
    return assemble_out(res.results)


# revision 15
# speedup vs baseline: 1.0110x; 1.0110x over previous
"""Multi-head self-attention (B=4, T=2048, D=1024, H=16) on 8 TRN2 NeuronCores.

Reference quirk: softmax normalizes over the QUERY axis (dim=2 of
[B,H,T1,T2]), i.e. attn[q,k] = exp(s[q,k]) / sum_q' exp(s[q',k]).

Sharding (fully SPMD, one NEFF for all 8 cores):
  core c -> batch b = c//2, head-group g = c%2 (8 heads = 512 cols of Wq/Wk/Wv).
  Host pre-slices AND pre-transposes per-core inputs (xT, wqT/wkT/wvT), runs
  the kernel, and stitches the 8 transposed [E, T] output shards back.

Device algorithm per core:
  1. QT/KT [128e, T] per head-pair (partition = head dims of 2 heads),
     V [128t, 512e] natural, via PE from xT / w*T tiles (fp32r).
  2. Per head-pair, per 128-wide key chunk:
       S' = K @ Q^T chunk [128 k, T q] in PSUM (row-tiled pair: head A rows
       0-63, head B rows 64-127, concurrent).
       exp: 3 of 4 [128,1024] tiles via ScalarE ACT (accum_out = Z partials);
       the 4th tile (head A, qb0) via the Vector engine: Schraudolph bit-trick
       (tensor_scalar fp32->int32 round-convert) + a custom 7-stage DVE op
       that polynomial-corrects the mantissa (max rel err ~0.54%) and
       accumulates the Z partial. This offloads ~25% of the exp stream from
       the Scalar engine (the serial bottleneck).
       V'[k,:] = V[k,:] / Z[k] (normalization folded into V, scaled on gpsimd),
       outT[d, q] += matmul: lhsT=V'_h [128k, 64d] (col tile_position per
       head, no zero padding), rhs=P [128k, 512q], accumulated in PSUM.
  3. Epilogue: acc -> SBUF -> DRAM as outT [E, T]; final transpose on host.

Pipeline: per chunk the PE stream is S -> projection fillers -> AV, so the PE
never idles long enough for the HAM clock gate to re-throttle.
"""

import operator

import numpy as np

B, T, D, H = 4, 2048, 1024, 16
DH = D // H
SCALE = 1.0 / (DH**0.5)
N_CORES = 8
E = D // 2  # 512 output cols per core (8 heads)
N_PAIRS = 4  # head-pairs per core
N_DC = D // 128  # 8 contraction chunks for projections
N_KC = T // 128  # 16 key chunks
QB = 1024  # exp free-dim block (2 PSUM banks)

# Schraudolph exp constants: z = round(2^23 * (log2e*SCALE*s + 127)),
# bitcast to fp32 gives 2^i*(1+f); custom DVE op multiplies by
# (1 + QC*f*(f-1)) to correct the mantissa interpolation.
A_SCHRAUD = float((2.0**23) * np.log2(np.e) * SCALE)
B_SCHRAUD = 127.0 * (2.0**23)
MASK_VAL = float(np.int32(0x007FFFFF).view(np.float32))  # +subnormal mantissa mask
TWO_P126 = float(np.float32(2.0**126))
QC = 0.23547743862603948

_built = None  # (nc,) cache so repeat kernel() calls skip rebuild/recompile
_exp_op = None


def _np_reference(x, padding_mask, Wq, Wk, Wv):
    """Pure-numpy fallback, used only if the mask is not all-ones."""
    x64 = x.astype(np.float64)
    Q = (x64 @ Wq.T.astype(np.float64)).reshape(B, T, H, DH).transpose(0, 2, 1, 3)
    K = (x64 @ Wk.T.astype(np.float64)).reshape(B, T, H, DH).transpose(0, 2, 1, 3)
    V = (x64 @ Wv.T.astype(np.float64)).reshape(B, T, H, DH).transpose(0, 2, 1, 3)
    s = np.einsum("bhqd,bhkd->bhqk", Q, K) * SCALE
    s = np.where(padding_mask[:, None, :, :] == 0, -np.inf, s)
    s = s - s.max(axis=2, keepdims=True)
    p = np.exp(s)
    p = p / p.sum(axis=2, keepdims=True)
    out = np.einsum("bhqk,bhkd->bhqd", p, V)
    return out.transpose(0, 2, 1, 3).reshape(B, T, D).astype(np.float32)


def _get_exp_op():
    """Register (once) the custom DVE op: out = z0*(1 + QC*f*(f-1)) with
    f extracted from z0's mantissa bits, plus a running sum (accum_out)."""
    global _exp_op
    if _exp_op is not None:
        return _exp_op
    import concourse.dve_ops as dve_ops_mod
    from concourse.dve_ops import DveOp
    from concourse.dve_spec import AluOp, Bin, C0, C1, C2, One, Spec, Src0, Zero, lower
    from concourse.dve_uop import DveOpSpec

    _a = Bin(AluOp.BITWISE_AND, Src0, C0)  # mantissa bits as +subnormal
    _d = _a * C1  # f in [0,1)  (subnormal * 2^126)
    _e = _d - One
    _u = _d * _e
    _v = _u * C2
    _t = _v * Src0
    spec = Spec(body=Src0 + _t, accum=operator.add, accum_init=Zero)

    name = "EXP_SFIX_ANT"
    existing = next((o for o in dve_ops_mod.OPS if o.name == name), None)
    if existing is not None:
        _exp_op = existing
        return existing
    sha = DveOpSpec(name=name, opcode=0, uops=lower(spec, ver="v3"), rd1_en=False).sha(
        "v3"
    )
    op = DveOp(name, spec, subdim=False, uops_sha={"v3": sha})
    dve_ops_mod.OPS.append(op)
    dve_ops_mod._SUB_OPCODE_FOR_NAME[name] = (
        dve_ops_mod._CUSTOM_DVE_ROW_BASE + len(dve_ops_mod.OPS) - 1
    )
    _exp_op = op
    return op


def _split_multi_waits(nc):
    """Walrus caps sync waits at 1 per instruction; Tile's tail drain can carry
    several. Move the extras onto single-wait drains appended to the previous
    basic block (same engine, earlier in program order)."""
    import concourse.mybir as mybir

    blocks = list(nc.m.functions[0].blocks)
    for bi, blk in enumerate(blocks):
        for inst in blk.instructions:
            if type(inst).__name__ not in ("InstDrain", "InstNoOp", "InstEventSemaphore"):
                continue
            si = inst.sync_info
            if si is not None and si.on_wait and len(si.on_wait) > 1:
                waits = list(si.on_wait)
                keep, extra = waits[-1], waits[:-1]
                assert all(w.wait_mode == "sem-ge-imm" for w in extra), extra
                si.on_wait = [keep]
                assert bi > 0, "multi-wait in first block"
                prev = blocks[bi - 1]
                for j, w in enumerate(extra):
                    d = mybir.InstDrain(
                        name=f"{inst.name}-ws{j}",
                        engine=inst.engine,
                        sync_info=mybir.SyncInfo(on_wait=[w], on_update=[]),
                    )
                    prev.add_instruction(d)


def _build_kernel(tc, xT, wqT, wkT, wvT, outT):
    import concourse.bass as bass  # noqa: F401
    import concourse.mybir as mybir

    nc = tc.nc
    FP = mybir.dt.float32
    FR = mybir.dt.float32r
    BF = mybir.dt.bfloat16
    I32 = mybir.dt.int32
    Exp = mybir.ActivationFunctionType.Exp
    exp_op = _get_exp_op()

    # long-lived pools
    xw = tc.alloc_tile_pool(name="xw", bufs=1)
    wp = tc.alloc_tile_pool(name="wp", bufs=3)
    qkv = tc.alloc_tile_pool(name="qkv", bufs=1)
    # PSUM: S pool (2x [128,QB] = 4 banks) shared by scores and projections;
    # acc pool (2x [128,QB] = 4 banks) for outT accumulation.
    sps = tc.alloc_tile_pool(name="sps", bufs=2, space="PSUM")
    accps = tc.alloc_tile_pool(name="accps", bufs=1, space="PSUM")
    pp = tc.alloc_tile_pool(name="pp", bufs=8)
    zp = tc.alloc_tile_pool(name="zp", bufs=4)
    z0p = tc.alloc_tile_pool(name="z0p", bufs=2)
    vpp = tc.alloc_tile_pool(name="vpp", bufs=4)
    op = tc.alloc_tile_pool(name="op", bufs=2)

    # ---- loads (interleaved so projection accumulation can start early) ----
    xTs = [None] * N_DC
    wq, wk, wv = [None] * N_DC, [None] * N_DC, [None] * N_DC
    for dc in range(N_DC):
        t = xw.tile([128, T], FR, name=f"xT{dc}", tag=f"x{dc}")
        nc.sync.dma_start(out=t, in_=xT[dc * 128 : (dc + 1) * 128, :])
        xTs[dc] = t
        for ws, wap, label in ((wq, wqT, "wq"), (wk, wkT, "wk"), (wv, wvT, "wv")):
            wt = wp.tile([128, E], FR, name=f"{label}{dc}", tag=f"w{dc}")
            nc.sync.dma_start(out=wt, in_=wap[dc * 128 : (dc + 1) * 128, :])
            ws[dc] = wt

    # ---- projection emitters (psum borrowed from the S pool tag) ----
    copy_flip = [0]
    ramp = [True]  # during the upfront ramp ScalarE is idle; share copies

    def _proj_copy(dst, src):
        """PSUM->SBUF projection copies: alternate ScalarE/VectorE during the
        upfront ramp (ACT idle there); VectorE only in steady state (ACT is
        the bottleneck engine then)."""
        if ramp[0] and copy_flip[0] % 2 == 0:
            nc.scalar.copy(dst, src)
        else:
            nc.vector.tensor_copy(dst, src)
        copy_flip[0] += 1

    def project_eT_tile(ws, pair, tt, et):
        """One [128, 512] t-block of QT/KT pair tile `et` (bf16 [128, T])."""
        ps = sps.tile([128, QB], FP, name=f"ps_{et.tensor.name}_{tt}", tag="s")
        for dc in range(N_DC):
            nc.tensor.matmul(
                ps[:, 0:512],
                ws[dc][:, pair * 128 : (pair + 1) * 128],
                xTs[dc][:, tt * 512 : (tt + 1) * 512],
                start=(dc == 0),
                stop=(dc == N_DC - 1),
            )
        _proj_copy(et[:, tt * 512 : (tt + 1) * 512], ps[:, 0:512])

    def project_v_tile(tt):
        v = qkv.tile([128, E], BF, name=f"v{tt}", tag=f"v{tt}")
        ps = sps.tile([128, QB], FP, name=f"ps_v{tt}", tag="s")
        for dc in range(N_DC):
            nc.tensor.matmul(
                ps[:, 0:512],
                xTs[dc][:, tt * 128 : (tt + 1) * 128],
                wv[dc],
                start=(dc == 0),
                stop=(dc == N_DC - 1),
            )
        _proj_copy(v, ps[:, 0:512])
        return v

    QT = [None] * N_PAIRS
    KT = [None] * N_PAIRS
    V = [None] * N_KC

    # pair-0 QT/KT + V[0] upfront (ramp); V[c] and later pairs' QT/KT spread
    # into the chunk stream as PE fillers (keeps the HAM clock gate warm).
    for pair in range(N_PAIRS):
        QT[pair] = qkv.tile([128, T], BF, name=f"qt{pair}", tag=f"qt{pair}")
        KT[pair] = qkv.tile([128, T], BF, name=f"kt{pair}", tag=f"kt{pair}")
    # Ramp is DMA-bound (~45us for x/w): everything emitted here executes
    # during the DMA wait for free. Cover pair 0 AND pair 1's QT/KT so the
    # steady-state filler load is V-only in pair 0, QK[p+1] in pairs 1-2.
    project_eT_tile(wk, 0, 0, KT[0])
    for tt in range(4):
        project_eT_tile(wq, 0, tt, QT[0])
    V[0] = project_v_tile(0)
    for tt in range(1, 4):
        project_eT_tile(wk, 0, tt, KT[0])
    for tt in range(4):
        project_eT_tile(wq, 1, tt, QT[1])
        project_eT_tile(wk, 1, tt, KT[1])
    ramp[0] = False

    # pair p+1's 8 QT/KT pieces spread over every other chunk of pair p
    FILLER_CHUNKS = list(range(0, 16, 2))

    for p in range(N_PAIRS):
        acc = [
            accps.tile([128, QB], FP, name=f"acc{qb}_{p}", tag=f"acc{qb}")
            for qb in range(2)
        ]
        pending_av = None
        for c in range(N_KC):
            kt_lo = KT[p][0:64, c * 128 : (c + 1) * 128]
            kt_hi = KT[p][64:128, c * 128 : (c + 1) * 128]

            def s_mm(dst, kt, base, q0):
                nc.tensor.matmul(
                    dst,
                    kt,
                    QT[p][base : base + 64, q0 : q0 + 512],
                    start=True,
                    stop=True,
                    tile_position=(base, 0),
                )

            def av_mm(hi, qb, qt, cc, vts_, pt_):
                nc.tensor.matmul(
                    acc[qb][:, qt * 512 : (qt + 1) * 512],
                    vts_[hi],
                    pt_[(hi, qb)][:, qt * 512 : (qt + 1) * 512],
                    start=(cc == 0 and hi == 0),
                    stop=(cc == N_KC - 1 and hi == 1),
                )

            # ---- scores qb0 (row-tiled concurrent pair) ----
            s0A = sps.tile([128, QB], FP, name=f"s_{p}_{c}_A0", tag="s")
            s0B = sps.tile([128, QB], FP, name=f"s_{p}_{c}_B0", tag="s")
            for qt in range(2):
                s_mm(s0A[:, qt * 512 : qt * 512 + 512], kt_lo, 0, qt * 512)
                s_mm(s0B[:, qt * 512 : qt * 512 + 512], kt_hi, 64, qt * 512)
            zs = zp.tile([128, 4], FP, name=f"zs_{p}_{c}", tag="zs")
            # head A qb0 -> Vector engine: Schraudolph int-convert (two halves
            # so s0A's PSUM buf frees as early as possible) + custom fixup.
            z0t = z0p.tile([128, QB], FP, name=f"z0_{p}_{c}", tag="z0")
            for h in range(2):
                nc.vector.tensor_scalar(
                    out=z0t.bitcast(I32)[:, h * 512 : (h + 1) * 512],
                    in0=s0A[:, h * 512 : (h + 1) * 512],
                    scalar1=A_SCHRAUD,
                    scalar2=B_SCHRAUD,
                    op0=mybir.AluOpType.mult,
                    op1=mybir.AluOpType.add,
                )
            pA0 = pp.tile([128, QB], BF, name=f"p_{p}_{c}_A0", tag="p")
            nc.vector._custom_dve(
                exp_op,
                out=pA0,
                in0=z0t,
                s0=MASK_VAL,
                s1=TWO_P126,
                imm2=QC,
                accum_out=zs[:, 0:1],
            )
            # head B qb0 -> Scalar engine
            pB0 = pp.tile([128, QB], BF, name=f"p_{p}_{c}_B0", tag="p")
            nc.scalar.activation(
                out=pB0, in_=s0B, func=Exp, scale=SCALE, accum_out=zs[:, 2:3]
            )
            # previous chunk's AV, first half (fills the PE while exp drains)
            if pending_av is not None:
                pc, pvts, ppt = pending_av
                for qt in range(2):
                    av_mm(0, 0, qt, pc, pvts, ppt)
                    av_mm(0, 1, qt, pc, pvts, ppt)
            # ---- scores qb1: B first (reuses s0A's buf, freed by op1) ----
            s1B = sps.tile([128, QB], FP, name=f"s_{p}_{c}_B1", tag="s")
            for qt in range(2):
                s_mm(s1B[:, qt * 512 : qt * 512 + 512], kt_hi, 64, QB + qt * 512)
            pB1 = pp.tile([128, QB], BF, name=f"p_{p}_{c}_B1", tag="p")
            nc.scalar.activation(
                out=pB1, in_=s1B, func=Exp, scale=SCALE, accum_out=zs[:, 3:4]
            )
            s1A = sps.tile([128, QB], FP, name=f"s_{p}_{c}_A1", tag="s")
            for qt in range(2):
                s_mm(s1A[:, qt * 512 : qt * 512 + 512], kt_lo, 0, QB + qt * 512)
            pA1 = pp.tile([128, QB], BF, name=f"p_{p}_{c}_A1", tag="p")
            nc.scalar.activation(
                out=pA1, in_=s1A, func=Exp, scale=SCALE, accum_out=zs[:, 1:2]
            )
            ptiles = {(0, 0): pA0, (0, 1): pA1, (1, 0): pB0, (1, 1): pB1}
            # previous chunk's AV, second half
            if pending_av is not None:
                pc, pvts, ppt = pending_av
                for qt in range(2):
                    av_mm(1, 0, qt, pc, pvts, ppt)
                    av_mm(1, 1, qt, pc, pvts, ppt)
            # ---- Z = qb0 + qb1 partial sums (gpsimd); r = 1/Z; V' = V*r ----
            za = zp.tile([128, 2], FP, name=f"za_{p}_{c}", tag="za")
            nc.gpsimd.tensor_add(za[:, 0:1], zs[:, 0:1], zs[:, 1:2])
            nc.gpsimd.tensor_add(za[:, 1:2], zs[:, 2:3], zs[:, 3:4])
            rz = zp.tile([128, 2], FP, name=f"rz_{p}_{c}", tag="rz")
            nc.vector.reciprocal(out=rz, in_=za)
            vts = []
            for hi in range(2):
                vt = vpp.tile([128, 128], BF, name=f"vp{hi}_{p}_{c}", tag=f"vp{hi}")
                lo, hi_ = (0, 64) if hi == 0 else (64, 128)
                zlo, zhi = (64, 128) if hi == 0 else (0, 64)
                nc.gpsimd.memset(vt[:, zlo:zhi], 0.0)
                nc.vector.tensor_scalar_mul(
                    vt[:, lo:hi_],
                    V[c][:, p * 128 + lo : p * 128 + hi_],
                    rz[:, hi : hi + 1],
                )
                vts.append(vt)
            pending_av = (c, vts, ptiles)
            # ---- fillers at slot end: their PSUM piece lands in the "s"
            # rotation after this chunk's 4 tiles; the copy runs early in the
            # next slot. ----
            if p == 0 and c + 1 < N_KC:
                V[c + 1] = project_v_tile(c + 1)
            if 1 <= p < N_PAIRS - 1 and c in FILLER_CHUNKS:
                idx = FILLER_CHUNKS.index(c)
                if idx < 4:
                    project_eT_tile(wq, p + 1, idx, QT[p + 1])
                else:
                    project_eT_tile(wk, p + 1, idx - 4, KT[p + 1])
        pc, pvts, ppt = pending_av
        for hi in range(2):
            for qt in range(2):
                av_mm(hi, 0, qt, pc, pvts, ppt)
                av_mm(hi, 1, qt, pc, pvts, ppt)
        # epilogue: outT rows for this pair -> SBUF -> DRAM (host transposes)
        for qb in range(2):
            ot = op.tile([128, QB], FP, name=f"ot_{p}_{qb}", tag="ot")
            nc.vector.tensor_copy(ot, acc[qb])
            nc.sync.dma_start(
                out=outT[p * 128 : (p + 1) * 128, qb * QB : (qb + 1) * QB],
                in_=ot,
            )

    for pool in (op, vpp, z0p, zp, pp, accps, sps, qkv, wp, xw):
        pool.release()


def build():
    import concourse.bacc as bacc
    import concourse.mybir as mybir
    import concourse.tile as tile

    nc = bacc.Bacc("TRN2", target_bir_lowering=False, debug=False)
    FP = mybir.dt.float32
    FR = mybir.dt.float32r
    xT = nc.dram_tensor("xT", [D, T], FR, kind="ExternalInput").ap()
    wqT = nc.dram_tensor("wqT", [D, E], FR, kind="ExternalInput").ap()
    wkT = nc.dram_tensor("wkT", [D, E], FR, kind="ExternalInput").ap()
    wvT = nc.dram_tensor("wvT", [D, E], FR, kind="ExternalInput").ap()
    outT = nc.dram_tensor("outT", [E, T], FP, kind="ExternalOutput").ap()
    with tile.TileContext(nc) as tc:
        _build_kernel(tc, xT, wqT, wkT, wvT, outT)
    nc.compile()
    _split_multi_waits(nc)
    return nc


def _get_nc():
    global _built
    if _built is None:
        _built = build()
    return _built


def make_in_maps(x, Wq, Wk, Wv):
    in_maps = []
    for c in range(N_CORES):
        b, g = divmod(c, 2)
        e0 = E * g
        in_maps.append(
            {
                "xT": np.ascontiguousarray(x[b].T),
                "wqT": np.ascontiguousarray(Wq[e0 : e0 + E, :].T),
                "wkT": np.ascontiguousarray(Wk[e0 : e0 + E, :].T),
                "wvT": np.ascontiguousarray(Wv[e0 : e0 + E, :].T),
            }
        )
    return in_maps


def assemble_out(results):
    out = np.empty((B, T, D), np.float32)
    for c in range(N_CORES):
        b, g = divmod(c, 2)
        e0 = E * g
        out[b][:, e0 : e0 + E] = results[c]["outT"].T
    return out


def kernel(x, padding_mask, Wq, Wk, Wv):
    x = np.asarray(x, dtype=np.float32)
    padding_mask = np.asarray(padding_mask, dtype=np.float32)
    Wq = np.asarray(Wq, dtype=np.float32)
    Wk = np.asarray(Wk, dtype=np.float32)
    Wv = np.asarray(Wv, dtype=np.float32)
    if not np.all(padding_mask == 1.0):
        return _np_reference(x, padding_mask, Wq, Wk, Wv)

    from concourse.bass_utils import run_bass_kernel_spmd

    nc = _get_nc()
    in_maps = make_in_maps(x, Wq, Wk, Wv)
    res = run_bass_kernel_spmd(nc, in_maps, list(range(N_CORES)))
    return assemble_out(res.results)


# revision 17
# speedup vs baseline: 1.0408x; 1.0294x over previous
"""Multi-head self-attention (B=4, T=2048, D=1024, H=16) on 8 TRN2 NeuronCores.

Reference quirk: softmax normalizes over the QUERY axis (dim=2 of
[B,H,T1,T2]), i.e. attn[q,k] = exp(s[q,k]) / sum_q' exp(s[q',k]).

Sharding (fully SPMD, one NEFF for all 8 cores):
  core c -> batch b = c//2, head-group g = c%2 (8 heads = 512 cols of Wq/Wk/Wv).
  Host pre-slices AND pre-transposes per-core inputs (xT, wqT/wkT/wvT), runs
  the kernel, and stitches the 8 transposed [E, T] output shards back.

Device algorithm per core:
  1. QT/KT [128e, T] per head-pair (partition = head dims of 2 heads),
     V [128t, 512e] natural, via PE from xT / w*T tiles (fp32r).
  2. Per head-pair, per 128-wide key chunk:
       S' = K @ Q^T chunk [128 k, T q] in PSUM (row-tiled pair: head A rows
       0-63, head B rows 64-127, concurrent).
       exp: 3 of 4 [128,1024] tiles via ScalarE ACT (accum_out = Z partials);
       the 4th tile (head A, qb0) via the Vector engine: Schraudolph bit-trick
       (tensor_scalar fp32->int32 round-convert) + a custom 7-stage DVE op
       that polynomial-corrects the mantissa (max rel err ~0.54%) and
       accumulates the Z partial. This offloads ~25% of the exp stream from
       the Scalar engine (the serial bottleneck).
       V'[k,:] = V[k,:] / Z[k] (normalization folded into V, scaled on gpsimd),
       outT[d, q] += matmul: lhsT=V'_h [128k, 64d] (col tile_position per
       head, no zero padding), rhs=P [128k, 512q], accumulated in PSUM.
  3. Epilogue: acc -> SBUF -> DRAM as outT [E, T]; final transpose on host.

Pipeline: per chunk the PE stream is S -> projection fillers -> AV, so the PE
never idles long enough for the HAM clock gate to re-throttle.
"""

import operator

import numpy as np

B, T, D, H = 4, 2048, 1024, 16
DH = D // H
SCALE = 1.0 / (DH**0.5)
N_CORES = 8
E = D // 2  # 512 output cols per core (8 heads)
N_PAIRS = 4  # head-pairs per core
N_DC = D // 128  # 8 contraction chunks for projections
N_KC = T // 128  # 16 key chunks
QB = 1024  # exp free-dim block (2 PSUM banks)

# Schraudolph exp constants: z = round(2^23 * (log2e*SCALE*s + 127)),
# bitcast to fp32 gives 2^i*(1+f); custom DVE op multiplies by
# (1 + QC*f*(f-1)) to correct the mantissa interpolation.
A_SCHRAUD = float((2.0**23) * np.log2(np.e) * SCALE)
B_SCHRAUD = 127.0 * (2.0**23)
MASK_VAL = float(np.int32(0x007FFFFF).view(np.float32))  # +subnormal mantissa mask
TWO_P126 = float(np.float32(2.0**126))
QC = 0.23547743862603948

_built = None  # (nc,) cache so repeat kernel() calls skip rebuild/recompile
_exp_op = None


def _np_reference(x, padding_mask, Wq, Wk, Wv):
    """Pure-numpy fallback, used only if the mask is not all-ones."""
    x64 = x.astype(np.float64)
    Q = (x64 @ Wq.T.astype(np.float64)).reshape(B, T, H, DH).transpose(0, 2, 1, 3)
    K = (x64 @ Wk.T.astype(np.float64)).reshape(B, T, H, DH).transpose(0, 2, 1, 3)
    V = (x64 @ Wv.T.astype(np.float64)).reshape(B, T, H, DH).transpose(0, 2, 1, 3)
    s = np.einsum("bhqd,bhkd->bhqk", Q, K) * SCALE
    s = np.where(padding_mask[:, None, :, :] == 0, -np.inf, s)
    s = s - s.max(axis=2, keepdims=True)
    p = np.exp(s)
    p = p / p.sum(axis=2, keepdims=True)
    out = np.einsum("bhqk,bhkd->bhqd", p, V)
    return out.transpose(0, 2, 1, 3).reshape(B, T, D).astype(np.float32)


def _get_exp_op():
    """Register (once) the custom DVE op: out = z0*(1 + QC*f*(f-1)) with
    f extracted from z0's mantissa bits, plus a running sum (accum_out)."""
    global _exp_op
    if _exp_op is not None:
        return _exp_op
    import concourse.dve_ops as dve_ops_mod
    from concourse.dve_ops import DveOp
    from concourse.dve_spec import AluOp, Bin, C0, C1, C2, One, Spec, Src0, Zero, lower
    from concourse.dve_uop import DveOpSpec

    _a = Bin(AluOp.BITWISE_AND, Src0, C0)  # mantissa bits as +subnormal
    _d = _a * C1  # f in [0,1)  (subnormal * 2^126)
    _e = _d - One
    _u = _d * _e
    _v = _u * C2
    _t = _v * Src0
    spec = Spec(body=Src0 + _t, accum=operator.add, accum_init=Zero)

    name = "EXP_SFIX_ANT"
    existing = next((o for o in dve_ops_mod.OPS if o.name == name), None)
    if existing is not None:
        _exp_op = existing
        return existing
    sha = DveOpSpec(name=name, opcode=0, uops=lower(spec, ver="v3"), rd1_en=False).sha(
        "v3"
    )
    op = DveOp(name, spec, subdim=False, uops_sha={"v3": sha})
    dve_ops_mod.OPS.append(op)
    dve_ops_mod._SUB_OPCODE_FOR_NAME[name] = (
        dve_ops_mod._CUSTOM_DVE_ROW_BASE + len(dve_ops_mod.OPS) - 1
    )
    _exp_op = op
    return op


def _split_multi_waits(nc):
    """Walrus caps sync waits at 1 per instruction; Tile's tail drain can carry
    several. Move the extras onto single-wait drains appended to the previous
    basic block (same engine, earlier in program order)."""
    import concourse.mybir as mybir

    blocks = list(nc.m.functions[0].blocks)
    for bi, blk in enumerate(blocks):
        for inst in blk.instructions:
            if type(inst).__name__ not in ("InstDrain", "InstNoOp", "InstEventSemaphore"):
                continue
            si = inst.sync_info
            if si is not None and si.on_wait and len(si.on_wait) > 1:
                waits = list(si.on_wait)
                keep, extra = waits[-1], waits[:-1]
                assert all(w.wait_mode == "sem-ge-imm" for w in extra), extra
                si.on_wait = [keep]
                assert bi > 0, "multi-wait in first block"
                prev = blocks[bi - 1]
                for j, w in enumerate(extra):
                    d = mybir.InstDrain(
                        name=f"{inst.name}-ws{j}",
                        engine=inst.engine,
                        sync_info=mybir.SyncInfo(on_wait=[w], on_update=[]),
                    )
                    prev.add_instruction(d)


def _build_kernel(tc, xT, wqT, wkT, wvT, outT):
    import concourse.bass as bass  # noqa: F401
    import concourse.mybir as mybir

    nc = tc.nc
    FP = mybir.dt.float32
    FR = mybir.dt.float32r
    BF = mybir.dt.bfloat16
    I32 = mybir.dt.int32
    Exp = mybir.ActivationFunctionType.Exp
    exp_op = _get_exp_op()

    # long-lived pools
    xw = tc.alloc_tile_pool(name="xw", bufs=1)
    wp = tc.alloc_tile_pool(name="wp", bufs=3)
    qkv = tc.alloc_tile_pool(name="qkv", bufs=1)
    # PSUM: S pool (2x [128,QB] = 4 banks) shared by scores and projections;
    # acc pool (2x [128,QB] = 4 banks) for outT accumulation.
    sps = tc.alloc_tile_pool(name="sps", bufs=2, space="PSUM")
    accps = tc.alloc_tile_pool(name="accps", bufs=1, space="PSUM")
    pp = tc.alloc_tile_pool(name="pp", bufs=8)
    zp = tc.alloc_tile_pool(name="zp", bufs=4)
    z0p = tc.alloc_tile_pool(name="z0p", bufs=2)
    vpp = tc.alloc_tile_pool(name="vpp", bufs=4)
    op = tc.alloc_tile_pool(name="op", bufs=2)

    # ---- loads (interleaved so projection accumulation can start early) ----
    xTs = [None] * N_DC
    wq, wk, wv = [None] * N_DC, [None] * N_DC, [None] * N_DC
    for dc in range(N_DC):
        t = xw.tile([128, T], FR, name=f"xT{dc}", tag=f"x{dc}")
        nc.sync.dma_start(out=t, in_=xT[dc * 128 : (dc + 1) * 128, :])
        xTs[dc] = t
        for ws, wap, label in ((wq, wqT, "wq"), (wk, wkT, "wk"), (wv, wvT, "wv")):
            wt = wp.tile([128, E], FR, name=f"{label}{dc}", tag=f"w{dc}")
            nc.sync.dma_start(out=wt, in_=wap[dc * 128 : (dc + 1) * 128, :])
            ws[dc] = wt

    # ---- projection emitters (psum borrowed from the S pool tag) ----
    copy_flip = [0]
    ramp = [True]  # during the upfront ramp ScalarE is idle; share copies

    def _proj_copy(dst, src):
        """PSUM->SBUF projection copies: alternate ScalarE/VectorE during the
        upfront ramp (ACT idle there); VectorE only in steady state (ACT is
        the bottleneck engine then)."""
        if ramp[0] and copy_flip[0] % 2 == 0:
            nc.scalar.copy(dst, src)
        else:
            nc.vector.tensor_copy(dst, src)
        copy_flip[0] += 1

    def project_eT_tile(ws, pair, tt, et):
        """One [128, 512] t-block of QT/KT pair tile `et` (bf16 [128, T])."""
        ps = sps.tile([128, QB], FP, name=f"ps_{et.tensor.name}_{tt}", tag="s")
        for dc in range(N_DC):
            nc.tensor.matmul(
                ps[:, 0:512],
                ws[dc][:, pair * 128 : (pair + 1) * 128],
                xTs[dc][:, tt * 512 : (tt + 1) * 512],
                start=(dc == 0),
                stop=(dc == N_DC - 1),
            )
        _proj_copy(et[:, tt * 512 : (tt + 1) * 512], ps[:, 0:512])

    def project_v_tile(tt):
        v = qkv.tile([128, E], BF, name=f"v{tt}", tag=f"v{tt}")
        ps = sps.tile([128, QB], FP, name=f"ps_v{tt}", tag="s")
        for dc in range(N_DC):
            nc.tensor.matmul(
                ps[:, 0:512],
                xTs[dc][:, tt * 128 : (tt + 1) * 128],
                wv[dc],
                start=(dc == 0),
                stop=(dc == N_DC - 1),
            )
        _proj_copy(v, ps[:, 0:512])
        return v

    QT = [None] * N_PAIRS
    KT = [None] * N_PAIRS
    V = [None] * N_KC

    # pair-0 QT/KT + V[0] upfront (ramp); V[c] and later pairs' QT/KT spread
    # into the chunk stream as PE fillers (keeps the HAM clock gate warm).
    for pair in range(N_PAIRS):
        QT[pair] = qkv.tile([128, T], BF, name=f"qt{pair}", tag=f"qt{pair}")
        KT[pair] = qkv.tile([128, T], BF, name=f"kt{pair}", tag=f"kt{pair}")
    # Ramp (DMA-gated): chunk-0-critical pieces first (KT0-tt0, QT0, V0),
    # then pair-1's early-deadline pieces. Late-deadline pieces (KT tails:
    # KT[p] piece tt isn't needed until pair p's chunk 4*tt) spread into the
    # chunk stream as PE fillers with per-pair schedules below.
    project_eT_tile(wk, 0, 0, KT[0])
    for tt in range(4):
        project_eT_tile(wq, 0, tt, QT[0])
    V[0] = project_v_tile(0)
    V[1] = project_v_tile(1)
    for tt in range(4):
        project_eT_tile(wq, 1, tt, QT[1])
    project_eT_tile(wk, 1, 0, KT[1])
    ramp[0] = False

    def emit_filler(p, c):
        if p == 0:
            if c < 3:  # KT0 piece tt (needed by chunk 4*tt)
                project_eT_tile(wk, 0, c + 1, KT[0])
            if c + 2 < N_KC:
                V[c + 2] = project_v_tile(c + 2)
        elif p < N_PAIRS:
            # KT[p] tail pieces early (deadline: own chunk 4*tt), then the
            # NEXT pair's QT + KT-tt0 (deadline: pair p+1 start).
            if c < 3:
                project_eT_tile(wk, p, c + 1, KT[p])
            elif p < N_PAIRS - 1 and c in (4, 6, 8, 10):
                project_eT_tile(wq, p + 1, (c - 4) // 2, QT[p + 1])
            elif p < N_PAIRS - 1 and c == 12:
                project_eT_tile(wk, p + 1, 0, KT[p + 1])

    for p in range(N_PAIRS):
        acc = [
            accps.tile([128, QB], FP, name=f"acc{qb}_{p}", tag=f"acc{qb}")
            for qb in range(2)
        ]
        pending_av = None
        for c in range(N_KC):
            kt_lo = KT[p][0:64, c * 128 : (c + 1) * 128]
            kt_hi = KT[p][64:128, c * 128 : (c + 1) * 128]

            def s_mm(dst, kt, base, q0):
                nc.tensor.matmul(
                    dst,
                    kt,
                    QT[p][base : base + 64, q0 : q0 + 512],
                    start=True,
                    stop=True,
                    tile_position=(base, 0),
                )

            def av_mm(hi, qb, qt, cc, vts_, pt_):
                nc.tensor.matmul(
                    acc[qb][:, qt * 512 : (qt + 1) * 512],
                    vts_[hi],
                    pt_[(hi, qb)][:, qt * 512 : (qt + 1) * 512],
                    start=(cc == 0 and hi == 0),
                    stop=(cc == N_KC - 1 and hi == 1),
                )

            # ---- scores qb0 (row-tiled concurrent pair) ----
            s0A = sps.tile([128, QB], FP, name=f"s_{p}_{c}_A0", tag="s")
            s0B = sps.tile([128, QB], FP, name=f"s_{p}_{c}_B0", tag="s")
            for qt in range(2):
                s_mm(s0A[:, qt * 512 : qt * 512 + 512], kt_lo, 0, qt * 512)
                s_mm(s0B[:, qt * 512 : qt * 512 + 512], kt_hi, 64, qt * 512)
            zs = zp.tile([128, 4], FP, name=f"zs_{p}_{c}", tag="zs")
            # head A qb0 -> Vector engine: Schraudolph int-convert (two halves
            # so s0A's PSUM buf frees as early as possible) + custom fixup.
            z0t = z0p.tile([128, QB], FP, name=f"z0_{p}_{c}", tag="z0")
            for h in range(2):
                nc.vector.tensor_scalar(
                    out=z0t.bitcast(I32)[:, h * 512 : (h + 1) * 512],
                    in0=s0A[:, h * 512 : (h + 1) * 512],
                    scalar1=A_SCHRAUD,
                    scalar2=B_SCHRAUD,
                    op0=mybir.AluOpType.mult,
                    op1=mybir.AluOpType.add,
                )
            pA0 = pp.tile([128, QB], BF, name=f"p_{p}_{c}_A0", tag="p")
            nc.vector._custom_dve(
                exp_op,
                out=pA0,
                in0=z0t,
                s0=MASK_VAL,
                s1=TWO_P126,
                imm2=QC,
                accum_out=zs[:, 0:1],
            )
            # head B qb0 -> Scalar engine
            pB0 = pp.tile([128, QB], BF, name=f"p_{p}_{c}_B0", tag="p")
            nc.scalar.activation(
                out=pB0, in_=s0B, func=Exp, scale=SCALE, accum_out=zs[:, 2:3]
            )
            # previous chunk's AV, first half (fills the PE while exp drains)
            if pending_av is not None:
                pc, pvts, ppt = pending_av
                for qt in range(2):
                    av_mm(0, 0, qt, pc, pvts, ppt)
                    av_mm(0, 1, qt, pc, pvts, ppt)
            # ---- scores qb1: B first (reuses s0A's buf, freed by op1) ----
            s1B = sps.tile([128, QB], FP, name=f"s_{p}_{c}_B1", tag="s")
            for qt in range(2):
                s_mm(s1B[:, qt * 512 : qt * 512 + 512], kt_hi, 64, QB + qt * 512)
            pB1 = pp.tile([128, QB], BF, name=f"p_{p}_{c}_B1", tag="p")
            nc.scalar.activation(
                out=pB1, in_=s1B, func=Exp, scale=SCALE, accum_out=zs[:, 3:4]
            )
            s1A = sps.tile([128, QB], FP, name=f"s_{p}_{c}_A1", tag="s")
            for qt in range(2):
                s_mm(s1A[:, qt * 512 : qt * 512 + 512], kt_lo, 0, QB + qt * 512)
            pA1 = pp.tile([128, QB], BF, name=f"p_{p}_{c}_A1", tag="p")
            nc.scalar.activation(
                out=pA1, in_=s1A, func=Exp, scale=SCALE, accum_out=zs[:, 1:2]
            )
            ptiles = {(0, 0): pA0, (0, 1): pA1, (1, 0): pB0, (1, 1): pB1}
            # previous chunk's AV, second half
            if pending_av is not None:
                pc, pvts, ppt = pending_av
                for qt in range(2):
                    av_mm(1, 0, qt, pc, pvts, ppt)
                    av_mm(1, 1, qt, pc, pvts, ppt)
            # ---- Z = qb0 + qb1 partial sums (gpsimd); r = 1/Z; V' = V*r ----
            za = zp.tile([128, 2], FP, name=f"za_{p}_{c}", tag="za")
            nc.gpsimd.tensor_add(za[:, 0:1], zs[:, 0:1], zs[:, 1:2])
            nc.gpsimd.tensor_add(za[:, 1:2], zs[:, 2:3], zs[:, 3:4])
            rz = zp.tile([128, 2], FP, name=f"rz_{p}_{c}", tag="rz")
            nc.vector.reciprocal(out=rz, in_=za)
            vts = []
            for hi in range(2):
                vt = vpp.tile([128, 128], BF, name=f"vp{hi}_{p}_{c}", tag=f"vp{hi}")
                lo, hi_ = (0, 64) if hi == 0 else (64, 128)
                zlo, zhi = (64, 128) if hi == 0 else (0, 64)
                nc.gpsimd.memset(vt[:, zlo:zhi], 0.0)
                nc.vector.tensor_scalar_mul(
                    vt[:, lo:hi_],
                    V[c][:, p * 128 + lo : p * 128 + hi_],
                    rz[:, hi : hi + 1],
                )
                vts.append(vt)
            pending_av = (c, vts, ptiles)
            # ---- fillers at slot end: their PSUM piece lands in the "s"
            # rotation after this chunk's 4 tiles; the copy runs early in the
            # next slot. ----
            emit_filler(p, c)
        pc, pvts, ppt = pending_av
        for hi in range(2):
            for qt in range(2):
                av_mm(hi, 0, qt, pc, pvts, ppt)
                av_mm(hi, 1, qt, pc, pvts, ppt)
        # epilogue: outT rows for this pair -> SBUF -> DRAM (host transposes)
        for qb in range(2):
            ot = op.tile([128, QB], FP, name=f"ot_{p}_{qb}", tag="ot")
            nc.vector.tensor_copy(ot, acc[qb])
            nc.sync.dma_start(
                out=outT[p * 128 : (p + 1) * 128, qb * QB : (qb + 1) * QB],
                in_=ot,
            )

    for pool in (op, vpp, z0p, zp, pp, accps, sps, qkv, wp, xw):
        pool.release()


def build():
    import concourse.bacc as bacc
    import concourse.mybir as mybir
    import concourse.tile as tile

    nc = bacc.Bacc("TRN2", target_bir_lowering=False, debug=False)
    FP = mybir.dt.float32
    FR = mybir.dt.float32r
    xT = nc.dram_tensor("xT", [D, T], FR, kind="ExternalInput").ap()
    wqT = nc.dram_tensor("wqT", [D, E], FR, kind="ExternalInput").ap()
    wkT = nc.dram_tensor("wkT", [D, E], FR, kind="ExternalInput").ap()
    wvT = nc.dram_tensor("wvT", [D, E], FR, kind="ExternalInput").ap()
    outT = nc.dram_tensor("outT", [E, T], FP, kind="ExternalOutput").ap()
    with tile.TileContext(nc) as tc:
        _build_kernel(tc, xT, wqT, wkT, wvT, outT)
    nc.compile()
    _split_multi_waits(nc)
    return nc


def _get_nc():
    global _built
    if _built is None:
        _built = build()
    return _built


def make_in_maps(x, Wq, Wk, Wv):
    in_maps = []
    for c in range(N_CORES):
        b, g = divmod(c, 2)
        e0 = E * g
        in_maps.append(
            {
                "xT": np.ascontiguousarray(x[b].T),
                "wqT": np.ascontiguousarray(Wq[e0 : e0 + E, :].T),
                "wkT": np.ascontiguousarray(Wk[e0 : e0 + E, :].T),
                "wvT": np.ascontiguousarray(Wv[e0 : e0 + E, :].T),
            }
        )
    return in_maps


def assemble_out(results):
    out = np.empty((B, T, D), np.float32)
    for c in range(N_CORES):
        b, g = divmod(c, 2)
        e0 = E * g
        out[b][:, e0 : e0 + E] = results[c]["outT"].T
    return out


def kernel(x, padding_mask, Wq, Wk, Wv):
    x = np.asarray(x, dtype=np.float32)
    padding_mask = np.asarray(padding_mask, dtype=np.float32)
    Wq = np.asarray(Wq, dtype=np.float32)
    Wk = np.asarray(Wk, dtype=np.float32)
    Wv = np.asarray(Wv, dtype=np.float32)
    if not np.all(padding_mask == 1.0):
        return _np_reference(x, padding_mask, Wq, Wk, Wv)

    from concourse.bass_utils import run_bass_kernel_spmd

    nc = _get_nc()
    in_maps = make_in_maps(x, Wq, Wk, Wv)
    res = run_bass_kernel_spmd(nc, in_maps, list(range(N_CORES)))
    return assemble_out(res.results)


# revision 20
# speedup vs baseline: 1.0491x; 1.0080x over previous
"""Multi-head self-attention (B=4, T=2048, D=1024, H=16) on 8 TRN2 NeuronCores.

Reference quirk: softmax normalizes over the QUERY axis (dim=2 of
[B,H,T1,T2]), i.e. attn[q,k] = exp(s[q,k]) / sum_q' exp(s[q',k]).

Sharding (fully SPMD, one NEFF for all 8 cores):
  core c -> batch b = c//2, head-group g = c%2 (8 heads = 512 cols of Wq/Wk/Wv).
  Host pre-slices AND pre-transposes per-core inputs (xT, wqT/wkT/wvT), runs
  the kernel, and stitches the 8 transposed [E, T] output shards back.

Device algorithm per core:
  1. QT/KT [128e, T] per head-pair (partition = head dims of 2 heads),
     V [128t, 512e] natural, via PE from xT / w*T tiles (fp32r).
  2. Per head-pair, per 128-wide key chunk:
       S' = K @ Q^T chunk [128 k, T q] in PSUM (row-tiled pair: head A rows
       0-63, head B rows 64-127, concurrent).
       exp: 3 of 4 [128,1024] tiles via ScalarE ACT (accum_out = Z partials);
       the 4th tile (head A, qb0) via the Vector engine: Schraudolph bit-trick
       (tensor_scalar fp32->int32 round-convert) + a custom 7-stage DVE op
       that polynomial-corrects the mantissa (max rel err ~0.54%) and
       accumulates the Z partial. This offloads ~25% of the exp stream from
       the Scalar engine (the serial bottleneck).
       V'[k,:] = V[k,:] / Z[k] (Z partials summed on gpsimd, reciprocal on
       VectorE, fold into a zero-padded [128,128] V' pair tile),
       outT[d, q] += matmul: lhsT=V'_pad, rhs=P [128k, 512q], PSUM-accumulated.
  3. Epilogue: acc -> SBUF -> DRAM as outT [E, T]; final transpose on host.

Pipelining: AV is deferred one chunk (so it never waits on the Z chain),
qb1's B-tile scores are emitted first (their PSUM buf frees via the fast DVE
path), and projections fill the PE between score bursts with staggered
deadlines (ramp covers chunk-0-critical pieces and pair 1's QT).
"""

import operator

import numpy as np

B, T, D, H = 4, 2048, 1024, 16
DH = D // H
SCALE = 1.0 / (DH**0.5)
N_CORES = 8
E = D // 2  # 512 output cols per core (8 heads)
N_PAIRS = 4  # head-pairs per core
N_DC = D // 128  # 8 contraction chunks for projections
N_KC = T // 128  # 16 key chunks
QB = 1024  # exp free-dim block (2 PSUM banks)

# Schraudolph exp constants: z = round(2^23 * (log2e*SCALE*s + 127)),
# bitcast to fp32 gives 2^i*(1+f); custom DVE op multiplies by
# (1 + QC*f*(f-1)) to correct the mantissa interpolation.
A_SCHRAUD = float((2.0**23) * np.log2(np.e) * SCALE)
B_SCHRAUD = 127.0 * (2.0**23)
MASK_VAL = float(np.int32(0x007FFFFF).view(np.float32))  # +subnormal mantissa mask
TWO_P126 = float(np.float32(2.0**126))
QC = 0.23547743862603948

_built = None  # (nc,) cache so repeat kernel() calls skip rebuild/recompile
_exp_op = None


def _np_reference(x, padding_mask, Wq, Wk, Wv):
    """Pure-numpy fallback, used only if the mask is not all-ones."""
    x64 = x.astype(np.float64)
    Q = (x64 @ Wq.T.astype(np.float64)).reshape(B, T, H, DH).transpose(0, 2, 1, 3)
    K = (x64 @ Wk.T.astype(np.float64)).reshape(B, T, H, DH).transpose(0, 2, 1, 3)
    V = (x64 @ Wv.T.astype(np.float64)).reshape(B, T, H, DH).transpose(0, 2, 1, 3)
    s = np.einsum("bhqd,bhkd->bhqk", Q, K) * SCALE
    s = np.where(padding_mask[:, None, :, :] == 0, -np.inf, s)
    s = s - s.max(axis=2, keepdims=True)
    p = np.exp(s)
    p = p / p.sum(axis=2, keepdims=True)
    out = np.einsum("bhqk,bhkd->bhqd", p, V)
    return out.transpose(0, 2, 1, 3).reshape(B, T, D).astype(np.float32)


def _get_exp_op():
    """Register (once) the custom DVE op: out = z0*(1 + QC*f*(f-1)) with
    f extracted from z0's mantissa bits, plus a running sum (accum_out)."""
    global _exp_op
    if _exp_op is not None:
        return _exp_op
    import concourse.dve_ops as dve_ops_mod
    from concourse.dve_ops import DveOp
    from concourse.dve_spec import AluOp, Bin, C0, C1, C2, One, Spec, Src0, Zero, lower
    from concourse.dve_uop import DveOpSpec

    _a = Bin(AluOp.BITWISE_AND, Src0, C0)  # mantissa bits as +subnormal
    _d = _a * C1  # f in [0,1)  (subnormal * 2^126)
    _e = _d - One
    _u = _d * _e
    _v = _u * C2
    _t = _v * Src0
    spec = Spec(body=Src0 + _t, accum=operator.add, accum_init=Zero)

    name = "EXP_SFIX_ANT"
    existing = next((o for o in dve_ops_mod.OPS if o.name == name), None)
    if existing is not None:
        _exp_op = existing
        return existing
    sha = DveOpSpec(name=name, opcode=0, uops=lower(spec, ver="v3"), rd1_en=False).sha(
        "v3"
    )
    op = DveOp(name, spec, subdim=False, uops_sha={"v3": sha})
    dve_ops_mod.OPS.append(op)
    dve_ops_mod._SUB_OPCODE_FOR_NAME[name] = (
        dve_ops_mod._CUSTOM_DVE_ROW_BASE + len(dve_ops_mod.OPS) - 1
    )
    _exp_op = op
    return op


def _split_multi_waits(nc):
    """Walrus caps sync waits at 1 per instruction; Tile's tail drain can carry
    several. Move the extras onto single-wait drains appended to the previous
    basic block (same engine, earlier in program order)."""
    import concourse.mybir as mybir

    blocks = list(nc.m.functions[0].blocks)
    for bi, blk in enumerate(blocks):
        for inst in blk.instructions:
            if type(inst).__name__ not in ("InstDrain", "InstNoOp", "InstEventSemaphore"):
                continue
            si = inst.sync_info
            if si is not None and si.on_wait and len(si.on_wait) > 1:
                waits = list(si.on_wait)
                keep, extra = waits[-1], waits[:-1]
                assert all(w.wait_mode == "sem-ge-imm" for w in extra), extra
                si.on_wait = [keep]
                assert bi > 0, "multi-wait in first block"
                prev = blocks[bi - 1]
                for j, w in enumerate(extra):
                    d = mybir.InstDrain(
                        name=f"{inst.name}-ws{j}",
                        engine=inst.engine,
                        sync_info=mybir.SyncInfo(on_wait=[w], on_update=[]),
                    )
                    prev.add_instruction(d)


def _build_kernel(tc, xT, wqT, wkT, wvT, outT):
    import concourse.bass as bass  # noqa: F401
    import concourse.mybir as mybir

    nc = tc.nc
    FP = mybir.dt.float32
    FR = mybir.dt.float32r
    BF = mybir.dt.bfloat16
    I32 = mybir.dt.int32
    Exp = mybir.ActivationFunctionType.Exp
    exp_op = _get_exp_op()

    # long-lived pools
    xw = tc.alloc_tile_pool(name="xw", bufs=1)
    wp = tc.alloc_tile_pool(name="wp", bufs=3)
    qkv = tc.alloc_tile_pool(name="qkv", bufs=1)
    # PSUM: S pool (2x [128,QB] = 4 banks) shared by scores and projections;
    # acc pool (2x [128,QB] = 4 banks) for outT accumulation.
    sps = tc.alloc_tile_pool(name="sps", bufs=2, space="PSUM")
    accps = tc.alloc_tile_pool(name="accps", bufs=1, space="PSUM")
    pp = tc.alloc_tile_pool(name="pp", bufs=8)
    zp = tc.alloc_tile_pool(name="zp", bufs=4)
    z0p = tc.alloc_tile_pool(name="z0p", bufs=2)
    vpp = tc.alloc_tile_pool(name="vpp", bufs=4)
    op = tc.alloc_tile_pool(name="op", bufs=2)

    # ---- loads (interleaved so projection accumulation can start early) ----
    xTs = [None] * N_DC
    wq, wk, wv = [None] * N_DC, [None] * N_DC, [None] * N_DC
    for dc in range(N_DC):
        t = xw.tile([128, T], FR, name=f"xT{dc}", tag=f"x{dc}")
        nc.sync.dma_start(out=t, in_=xT[dc * 128 : (dc + 1) * 128, :])
        xTs[dc] = t
        for ws, wap, label in ((wq, wqT, "wq"), (wk, wkT, "wk"), (wv, wvT, "wv")):
            wt = wp.tile([128, E], FR, name=f"{label}{dc}", tag=f"w{dc}")
            nc.sync.dma_start(out=wt, in_=wap[dc * 128 : (dc + 1) * 128, :])
            ws[dc] = wt

    # ---- projection emitters (psum borrowed from the S pool tag) ----
    copy_flip = [0]
    ramp = [True]  # during the upfront ramp ScalarE is idle; share copies

    def _proj_copy(dst, src):
        """PSUM->SBUF projection copies: alternate ScalarE/VectorE during the
        upfront ramp (ACT idle there); VectorE only in steady state (ACT is
        the bottleneck engine then)."""
        if ramp[0] and copy_flip[0] % 2 == 0:
            nc.scalar.copy(dst, src)
        else:
            nc.vector.tensor_copy(dst, src)
        copy_flip[0] += 1

    def project_eT_tile(ws, pair, tt, et):
        """One [128, 512] t-block of QT/KT pair tile `et` (bf16 [128, T])."""
        ps = sps.tile([128, QB], FP, name=f"ps_{et.tensor.name}_{tt}", tag="s")
        for dc in range(N_DC):
            nc.tensor.matmul(
                ps[:, 0:512],
                ws[dc][:, pair * 128 : (pair + 1) * 128],
                xTs[dc][:, tt * 512 : (tt + 1) * 512],
                start=(dc == 0),
                stop=(dc == N_DC - 1),
            )
        _proj_copy(et[:, tt * 512 : (tt + 1) * 512], ps[:, 0:512])

    def project_v_tile(tt):
        v = qkv.tile([128, E], BF, name=f"v{tt}", tag=f"v{tt}")
        ps = sps.tile([128, QB], FP, name=f"ps_v{tt}", tag="s")
        for dc in range(N_DC):
            nc.tensor.matmul(
                ps[:, 0:512],
                xTs[dc][:, tt * 128 : (tt + 1) * 128],
                wv[dc],
                start=(dc == 0),
                stop=(dc == N_DC - 1),
            )
        _proj_copy(v, ps[:, 0:512])
        return v

    QT = [None] * N_PAIRS
    KT = [None] * N_PAIRS
    V = [None] * N_KC

    # pair-0 QT/KT + V[0] upfront (ramp); V[c] and later pairs' QT/KT spread
    # into the chunk stream as PE fillers (keeps the HAM clock gate warm).
    for pair in range(N_PAIRS):
        QT[pair] = qkv.tile([128, T], BF, name=f"qt{pair}", tag=f"qt{pair}")
        KT[pair] = qkv.tile([128, T], BF, name=f"kt{pair}", tag=f"kt{pair}")
    # Ramp (DMA-gated): chunk-0-critical pieces first (KT0-tt0, QT0, V0),
    # then pair-1's early-deadline pieces. Late-deadline pieces (KT tails:
    # KT[p] piece tt isn't needed until pair p's chunk 4*tt) spread into the
    # chunk stream as PE fillers with per-pair schedules below.
    project_eT_tile(wk, 0, 0, KT[0])
    for tt in range(4):
        project_eT_tile(wq, 0, tt, QT[0])
    V[0] = project_v_tile(0)
    V[1] = project_v_tile(1)
    for tt in range(4):
        project_eT_tile(wq, 1, tt, QT[1])
    project_eT_tile(wk, 1, 0, KT[1])
    ramp[0] = False

    def emit_filler(p, c):
        if p == 0:
            if c < 3:  # KT0 piece tt (needed by chunk 4*tt)
                project_eT_tile(wk, 0, c + 1, KT[0])
            if c + 2 < N_KC:
                V[c + 2] = project_v_tile(c + 2)
        elif p < N_PAIRS:
            # KT[p] tail pieces early (deadline: own chunk 4*tt), then the
            # NEXT pair's QT + KT-tt0 (deadline: pair p+1 start).
            if c < 3:
                project_eT_tile(wk, p, c + 1, KT[p])
            elif p < N_PAIRS - 1 and c in (4, 6, 8, 10):
                project_eT_tile(wq, p + 1, (c - 4) // 2, QT[p + 1])
            elif p < N_PAIRS - 1 and c == 12:
                project_eT_tile(wk, p + 1, 0, KT[p + 1])

    for p in range(N_PAIRS):
        acc = [
            accps.tile([128, QB], FP, name=f"acc{qb}_{p}", tag=f"acc{qb}")
            for qb in range(2)
        ]
        pending_av = None
        for c in range(N_KC):
            kt_lo = KT[p][0:64, c * 128 : (c + 1) * 128]
            kt_hi = KT[p][64:128, c * 128 : (c + 1) * 128]

            def s_mm(dst, kt, base, q0):
                nc.tensor.matmul(
                    dst,
                    kt,
                    QT[p][base : base + 64, q0 : q0 + 512],
                    start=True,
                    stop=True,
                    tile_position=(base, 0),
                )

            def av_mm(hi, qb, qt, cc, vts_, pt_):
                nc.tensor.matmul(
                    acc[qb][:, qt * 512 : (qt + 1) * 512],
                    vts_[hi],
                    pt_[(hi, qb)][:, qt * 512 : (qt + 1) * 512],
                    start=(cc == 0 and hi == 0),
                    stop=(cc == N_KC - 1 and hi == 1),
                )

            # ---- scores qb0 (row-tiled concurrent pair) ----
            s0A = sps.tile([128, QB], FP, name=f"s_{p}_{c}_A0", tag="s")
            s0B = sps.tile([128, QB], FP, name=f"s_{p}_{c}_B0", tag="s")
            for qt in range(2):
                s_mm(s0A[:, qt * 512 : qt * 512 + 512], kt_lo, 0, qt * 512)
                s_mm(s0B[:, qt * 512 : qt * 512 + 512], kt_hi, 64, qt * 512)
            zs = zp.tile([128, 4], FP, name=f"zs_{p}_{c}", tag="zs")
            # head A qb0 -> Vector engine: Schraudolph int-convert (two halves
            # so s0A's PSUM buf frees as early as possible) + custom fixup.
            z0t = z0p.tile([128, QB], FP, name=f"z0_{p}_{c}", tag="z0")
            for h in range(2):
                nc.vector.tensor_scalar(
                    out=z0t.bitcast(I32)[:, h * 512 : (h + 1) * 512],
                    in0=s0A[:, h * 512 : (h + 1) * 512],
                    scalar1=A_SCHRAUD,
                    scalar2=B_SCHRAUD,
                    op0=mybir.AluOpType.mult,
                    op1=mybir.AluOpType.add,
                )
            pA0 = pp.tile([128, QB], BF, name=f"p_{p}_{c}_A0", tag="p")
            nc.vector._custom_dve(
                exp_op,
                out=pA0,
                in0=z0t,
                s0=MASK_VAL,
                s1=TWO_P126,
                imm2=QC,
                accum_out=zs[:, 0:1],
            )
            # head B qb0 -> Scalar engine
            pB0 = pp.tile([128, QB], BF, name=f"p_{p}_{c}_B0", tag="p")
            nc.scalar.activation(
                out=pB0, in_=s0B, func=Exp, scale=SCALE, accum_out=zs[:, 2:3]
            )
            # previous chunk's AV, first half (fills the PE while exp drains)
            if pending_av is not None:
                pc, pvts, ppt = pending_av
                for qt in range(2):
                    av_mm(0, 0, qt, pc, pvts, ppt)
                    av_mm(0, 1, qt, pc, pvts, ppt)
            # ---- scores qb1: B first (reuses s0A's buf, freed by op1) ----
            s1B = sps.tile([128, QB], FP, name=f"s_{p}_{c}_B1", tag="s")
            for qt in range(2):
                s_mm(s1B[:, qt * 512 : qt * 512 + 512], kt_hi, 64, QB + qt * 512)
            pB1 = pp.tile([128, QB], BF, name=f"p_{p}_{c}_B1", tag="p")
            nc.scalar.activation(
                out=pB1, in_=s1B, func=Exp, scale=SCALE, accum_out=zs[:, 3:4]
            )
            s1A = sps.tile([128, QB], FP, name=f"s_{p}_{c}_A1", tag="s")
            for qt in range(2):
                s_mm(s1A[:, qt * 512 : qt * 512 + 512], kt_lo, 0, QB + qt * 512)
            pA1 = pp.tile([128, QB], BF, name=f"p_{p}_{c}_A1", tag="p")
            nc.scalar.activation(
                out=pA1, in_=s1A, func=Exp, scale=SCALE, accum_out=zs[:, 1:2]
            )
            ptiles = {(0, 0): pA0, (0, 1): pA1, (1, 0): pB0, (1, 1): pB1}
            # previous chunk's AV, second half
            if pending_av is not None:
                pc, pvts, ppt = pending_av
                for qt in range(2):
                    av_mm(1, 0, qt, pc, pvts, ppt)
                    av_mm(1, 1, qt, pc, pvts, ppt)
            # ---- Z = qb0 + qb1 partial sums (gpsimd); r = 1/Z; V' = V*r ----
            za = zp.tile([128, 2], FP, name=f"za_{p}_{c}", tag="za")
            nc.gpsimd.tensor_add(za[:, 0:1], zs[:, 0:1], zs[:, 1:2])
            nc.gpsimd.tensor_add(za[:, 1:2], zs[:, 2:3], zs[:, 3:4])
            rz = zp.tile([128, 2], FP, name=f"rz_{p}_{c}", tag="rz")
            nc.vector.reciprocal(out=rz, in_=za)
            vts = []
            for hi in range(2):
                vt = vpp.tile([128, 128], BF, name=f"vp{hi}_{p}_{c}", tag=f"vp{hi}")
                lo, hi_ = (0, 64) if hi == 0 else (64, 128)
                zlo, zhi = (64, 128) if hi == 0 else (0, 64)
                nc.gpsimd.memset(vt[:, zlo:zhi], 0.0)
                nc.vector.tensor_scalar_mul(
                    vt[:, lo:hi_],
                    V[c][:, p * 128 + lo : p * 128 + hi_],
                    rz[:, hi : hi + 1],
                )
                vts.append(vt)
            pending_av = (c, vts, ptiles)
            # ---- fillers at slot end: their PSUM piece lands in the "s"
            # rotation after this chunk's 4 tiles; the copy runs early in the
            # next slot. ----
            emit_filler(p, c)
        pc, pvts, ppt = pending_av
        for hi in range(2):
            for qt in range(2):
                av_mm(hi, 0, qt, pc, pvts, ppt)
                av_mm(hi, 1, qt, pc, pvts, ppt)
        # epilogue: outT rows for this pair -> SBUF -> DRAM (host transposes)
        for qb in range(2):
            ot = op.tile([128, QB], FP, name=f"ot_{p}_{qb}", tag="ot")
            nc.vector.tensor_copy(ot, acc[qb])
            nc.sync.dma_start(
                out=outT[p * 128 : (p + 1) * 128, qb * QB : (qb + 1) * QB],
                in_=ot,
            )

    for pool in (op, vpp, z0p, zp, pp, accps, sps, qkv, wp, xw):
        pool.release()


def build():
    import concourse.bacc as bacc
    import concourse.mybir as mybir
    import concourse.tile as tile

    nc = bacc.Bacc("TRN2", target_bir_lowering=False, debug=False)
    FP = mybir.dt.float32
    FR = mybir.dt.float32r
    xT = nc.dram_tensor("xT", [D, T], FR, kind="ExternalInput").ap()
    wqT = nc.dram_tensor("wqT", [D, E], FR, kind="ExternalInput").ap()
    wkT = nc.dram_tensor("wkT", [D, E], FR, kind="ExternalInput").ap()
    wvT = nc.dram_tensor("wvT", [D, E], FR, kind="ExternalInput").ap()
    outT = nc.dram_tensor("outT", [E, T], FP, kind="ExternalOutput").ap()
    with tile.TileContext(nc) as tc:
        _build_kernel(tc, xT, wqT, wkT, wvT, outT)
    nc.compile()
    _split_multi_waits(nc)
    return nc


def _get_nc():
    global _built
    if _built is None:
        _built = build()
    return _built


def make_in_maps(x, Wq, Wk, Wv):
    in_maps = []
    for c in range(N_CORES):
        b, g = divmod(c, 2)
        e0 = E * g
        in_maps.append(
            {
                "xT": np.ascontiguousarray(x[b].T),
                "wqT": np.ascontiguousarray(Wq[e0 : e0 + E, :].T),
                "wkT": np.ascontiguousarray(Wk[e0 : e0 + E, :].T),
                "wvT": np.ascontiguousarray(Wv[e0 : e0 + E, :].T),
            }
        )
    return in_maps


def assemble_out(results):
    out = np.empty((B, T, D), np.float32)
    for c in range(N_CORES):
        b, g = divmod(c, 2)
        e0 = E * g
        out[b][:, e0 : e0 + E] = results[c]["outT"].T
    return out


def kernel(x, padding_mask, Wq, Wk, Wv):
    x = np.asarray(x, dtype=np.float32)
    padding_mask = np.asarray(padding_mask, dtype=np.float32)
    Wq = np.asarray(Wq, dtype=np.float32)
    Wk = np.asarray(Wk, dtype=np.float32)
    Wv = np.asarray(Wv, dtype=np.float32)
    if not np.all(padding_mask == 1.0):
        return _np_reference(x, padding_mask, Wq, Wk, Wv)

    from concourse.bass_utils import run_bass_kernel_spmd

    nc = _get_nc()
    in_maps = make_in_maps(x, Wq, Wk, Wv)
    res = run_bass_kernel_spmd(nc, in_maps, list(range(N_CORES)))
    return assemble_out(res.results)
